# revision 3
# baseline (speedup 1.0000x reference)
"""Trainium2 Bass kernel: 4096x4096 image, 16x16 valid cross-correlation + bias.

Strategy: shard output rows across 8 NeuronCores (spatial parallel, halo rows
overlapped host-side). Per core, the conv is computed as banded matmuls on
TensorE.

v2 scheme (stride-128 strips + cross-strip completion):
  - Strips advance by 128 rows (no overlap). The stationary banded matrix is
    [128 x 128]: column m holds taps w[0..15, b] at rows m..m+15, truncated at
    row 127. Rows m <= 112 of each PSUM tile are complete after the 16 main
    passes; rows 113..127 are missing the taps that fall into the next strip.
  - Two "completion" passes per column tile finish rows 113..127: the first 15
    rows of the next strip are duplicated into a [120 = 8 shifts x 15 rows]
    buffer D[bp*15+r', c] = Xnext[r', c+bp] (built on the idle Scalar engine),
    so one matmul pass covers 8 kernel columns at once; h in {0,1} covers
    kernel columns bp+8h.
  - Per core: 4 strips x 8 column tiles x 18 passes = 576 matmuls (vs 640 for
    the stride-113 scheme) and the per-core row count 512 = 4*128 divides
    evenly, so all cores do identical work (SPMD-balanced).
  - Dummy matmuls at t=0 ramp the PE clock (HAM warmup) while the first strip
    DMA lands, so real matmuls run at full rate from the start.

Matmul dtype default bfloat16 (1 cycle/row, halves DMA traffic vs fp32;
l2 rel err ~4e-3, well under the 2e-2 gate). Output is stored bf16 and
upconverted to fp32 host-side.
"""
import os
import numpy as np

import concourse.mybir as mybir
import concourse.tile as tile
from concourse import bacc
from concourse.bass_utils import run_bass_kernel_spmd

H = 4096
W = 4096
KH = 16
KW = 16
OH = H - KH + 1  # 4081
OW = W - KW + 1  # 4081
NCORES = 8

RPC = 512  # output rows per core (8*512 = 4096 >= 4081; tail trimmed)
SSTRIDE = 128  # strip stride
NSTRIPS = RPC // SSTRIDE  # 4
HALO = KH - 1  # 15
IN_ROWS = RPC + HALO  # 527 input rows needed per core
N_TILE = 512  # output cols per matmul (PSUM bank limit for fp32)

# column tiles: 7 full + ragged last computed at n0 = OW - N_TILE, storing
# only the non-overlapping tail (skip = overlap with tile 6)
N0_LIST = [(n0, 0) for n0 in range(0, OW - N_TILE + 1, N_TILE)]
_covered = (OW // N_TILE) * N_TILE
if _covered < OW:
    N0_LIST.append((OW - N_TILE, _covered - (OW - N_TILE)))
DUP_W = (OW - N_TILE) + 8 + N_TILE  # 4089; max col streamed from dup buffer

_DT_NAME = os.environ.get("CONV_MM_DTYPE", "bfloat16")
_OUT_DT_NAME = os.environ.get("CONV_OUT_DTYPE", "bfloat16")

_build_cache = {}


def _build(dt_name):
    loop = int(os.environ.get("CONV_LOOP", "1"))  # hw For_i loop (bench only)
    warm = int(os.environ.get("CONV_WARMUP_MM", "16"))
    psum_bufs = int(os.environ.get("CONV_PSUM_BUFS", "8"))
    strip_bufs = int(os.environ.get("CONV_STRIP_BUFS", "3"))
    out_dt_name = _OUT_DT_NAME
    key = (dt_name, out_dt_name, loop, warm, psum_bufs, strip_bufs)
    if key in _build_cache:
        return _build_cache[key]
    DT = getattr(mybir.dt, dt_name)
    ODT = getattr(mybir.dt, out_dt_name)
    nc = bacc.Bacc()
    x_d = nc.dram_tensor("Xs", [IN_ROWS, W], DT, kind="ExternalInput")
    wb_d = nc.dram_tensor("wband", [128, KW, 128], DT, kind="ExternalInput")
    wc_d = nc.dram_tensor("wcomp", [120, 2, 128], DT, kind="ExternalInput")
    bias_d = nc.dram_tensor("biasb", [128, 1], mybir.dt.float32, kind="ExternalInput")
    out_d = nc.dram_tensor("out", [RPC, OW], ODT, kind="ExternalOutput")

    # strip DMA split points: chunk j ends where column tile j's reads end, so
    # tile j's matmuls only wait on chunks <= j
    chunk_bounds = [0]
    for j in range(len(N0_LIST)):
        chunk_bounds.append(min((j + 1) * N_TILE + KW - 1, W))

    with tile.TileContext(nc) as tc:
        with (
            tc.tile_pool(name="const", bufs=1) as cpool,
            tc.tile_pool(name="strip", bufs=strip_bufs) as spool,
            tc.tile_pool(name="head", bufs=NSTRIPS) as hpool,
            tc.tile_pool(name="dup", bufs=NSTRIPS) as dpool,
            tc.tile_pool(name="obuf", bufs=3) as opool,
            tc.tile_pool(name="psum", bufs=psum_bufs, space="PSUM") as ppool,
        ):
            # PE warmup: dummy matmuls on a zeroed tile ramp the PE clock
            # while the first strip DMA is in flight
            if warm:
                wsrc = cpool.tile([128, N_TILE], DT)
                nc.vector.memset(wsrc[:, :], 0.0)
                for _ in range(warm):
                    wp = ppool.tile([128, N_TILE], mybir.dt.float32, tag="ps")
                    nc.tensor.matmul(
                        wp[:, :], wsrc[:, :128], wsrc[:, :N_TILE],
                        start=True, stop=True,
                    )

            # constants: b=0 band slice first (gates the very first matmul);
            # bulk on the SWDGE queue so it doesn't delay strip chunks
            wb = cpool.tile([128, KW, 128], DT)
            nc.sync.dma_start(wb[:, 0:1, :], wb_d[:, 0:1, :])
            nc.gpsimd.dma_start(wb[:, 1:, :], wb_d[:, 1:, :])
            wcm = cpool.tile([120, 2, 128], DT)
            nc.gpsimd.dma_start(wcm[:], wc_d[:])
            bias_sb = cpool.tile([128, 1], mybir.dt.float32)
            nc.gpsimd.dma_start(bias_sb[:], bias_d[:])

            def body():
                # strip 0 chunked so the first matmuls aren't gated on 2MB
                strip0 = spool.tile([128, W], DT, tag="strip")
                for lo, hi in zip(chunk_bounds, chunk_bounds[1:]):
                    if hi > lo:
                        nc.sync.dma_start(strip0[:, lo:hi], x_d[0:128, lo:hi])

                # next-strip heads + shifted duplicate buffers (Scalar engine)
                dups = []
                for s in range(NSTRIPS):
                    base = SSTRIDE * (s + 1)
                    headt = hpool.tile([HALO, W], DT, tag="head")
                    nc.sync.dma_start(headt[:, :], x_d[base : base + HALO, :])
                    d = dpool.tile([120, DUP_W], DT, tag="dup")
                    for bp in range(8):
                        nc.scalar.copy(
                            d[bp * HALO : (bp + 1) * HALO, :],
                            headt[:, bp : bp + DUP_W],
                        )
                    dups.append(d)

                strips = [strip0]
                for s in range(1, NSTRIPS):
                    st = spool.tile([128, W], DT, tag="strip")
                    nc.sync.dma_start(st[:, :], x_d[SSTRIDE * s : SSTRIDE * s + 128, :])
                    strips.append(st)

                for s in range(NSTRIPS):
                    strip = strips[s]
                    dup = dups[s]
                    for n0, skip in N0_LIST:
                        ps = ppool.tile([128, N_TILE], mybir.dt.float32, tag="ps")
                        for b in range(KW):
                            nc.tensor.matmul(
                                ps[:, :],
                                wb[:, b, :],
                                strip[:, n0 + b : n0 + b + N_TILE],
                                start=(b == 0),
                                stop=False,
                            )
                        for h2 in range(2):
                            nc.tensor.matmul(
                                ps[:, :],
                                wcm[:, h2, :],
                                dup[:, n0 + 8 * h2 : n0 + 8 * h2 + N_TILE],
                                start=False,
                                stop=(h2 == 1),
                            )
                        nt = N_TILE - skip
                        ot = opool.tile([128, N_TILE], ODT, tag="ot")
                        nc.vector.tensor_scalar_add(
                            ot[:, :nt], ps[:, skip:N_TILE], bias_sb[:, :]
                        )
                        nc.sync.dma_start(
                            out_d[SSTRIDE * s : SSTRIDE * s + 128, n0 + skip : n0 + N_TILE],
                            ot[:, :nt],
                        )

            if loop > 1:
                with tc.For_i(0, loop, 1):
                    body()
            else:
                body()
    nc.finalize()
    _build_cache[key] = nc
    return nc


def make_in_maps(X, weight, bias, np_dt):
    pad_rows = NCORES * RPC + HALO  # 4111
    Xpad = np.zeros((pad_rows, W), dtype=np_dt)
    Xpad[:H] = X.astype(np_dt)

    wc = weight.astype(np_dt)
    # wband[r, b, m] = w[r - m, b] for 0 <= r-m < 16, truncated at r = 127
    wband = np.zeros((128, KW, 128), dtype=np_dt)
    for m in range(128):
        k = min(KH, 128 - m)
        wband[m : m + k, :, m] = wc[:k]
    # wcomp[bp*15 + r', h, m] = w[r' + 128 - m, bp + 8h] for taps falling in
    # the next strip (output rows m in [113, 127])
    wcomp = np.zeros((120, 2, 128), dtype=np_dt)
    for bp in range(8):
        for rp in range(HALO):
            p = bp * HALO + rp
            for h2 in range(2):
                b = bp + 8 * h2
                for m in range(rp + 113, 128):
                    wcomp[p, h2, m] = wc[rp + 128 - m, b]
    biasb = np.full((128, 1), np.float32(bias[0]), dtype=np.float32)

    return [
        {
            "Xs": np.ascontiguousarray(Xpad[c * RPC : c * RPC + IN_ROWS]),
            "wband": wband,
            "wcomp": wcomp,
            "biasb": biasb,
        }
        for c in range(NCORES)
    ]


def _run(X, weight, bias, dt_name, trace=False):
    nc = _build(dt_name)
    np_dt = mybir.dt.np(getattr(mybir.dt, dt_name))
    in_maps = make_in_maps(X, weight, bias, np_dt)
    res = run_bass_kernel_spmd(
        nc, in_maps, core_ids=list(range(NCORES)), trace=trace
    )
    out = np.concatenate([res.results[c]["out"] for c in range(NCORES)], axis=0)
    return np.asarray(out[:OH], dtype=np.float32), res


def kernel(X, weight, bias):
    X = np.asarray(X, dtype=np.float32)
    weight = np.asarray(weight, dtype=np.float32)
    bias = np.asarray(bias, dtype=np.float32)
    out, _ = _run(X, weight, bias, _DT_NAME, trace=False)
    return out


# revision 21
# speedup vs baseline: 1.3305x; 1.3305x over previous
"""Trainium2 Bass kernel: 4096x4096 image, 16x16 valid cross-correlation + bias.

Strategy: shard output rows across 8 NeuronCores (spatial parallel, halo rows
overlapped host-side). Per core, the conv is computed as banded matmuls on
TensorE.

v2 scheme (stride-128 strips + cross-strip completion):
  - Strips advance by 128 rows (no overlap). The stationary banded matrix is
    [128 x 128]: column m holds taps w[0..15, b] at rows m..m+15, truncated at
    row 127. Rows m <= 112 of each PSUM tile are complete after the 16 main
    passes; rows 113..127 are missing the taps that fall into the next strip.
  - Two "completion" passes per column tile finish rows 113..127: the first 15
    rows of the next strip are duplicated into a [120 = 8 shifts x 15 rows]
    buffer D[bp*15+r', c] = Xnext[r', c+bp] (built on the idle Scalar engine),
    so one matmul pass covers 8 kernel columns at once; h in {0,1} covers
    kernel columns bp+8h.
  - Per core: 4 strips x 8 column tiles x 18 passes = 576 matmuls (vs 640 for
    the stride-113 scheme) and the per-core row count 512 = 4*128 divides
    evenly, so all cores do identical work (SPMD-balanced).
  - Dummy matmuls at t=0 ramp the PE clock (HAM warmup) while the first strip
    DMA lands, so real matmuls run at full rate from the start.

Matmul dtype float32r (fp32 storage, ~13-bit-mantissa multiplies, 1 cycle/row
for N>=256 — measured faster than bf16 on this device, which pays an
alignment penalty on 2-byte moving operands). Output is stored bf16 and
upconverted to fp32 host-side (l2 rel err ~1e-3, well under the 2e-2 gate).
"""
import os
import numpy as np

import concourse.mybir as mybir
import concourse.tile as tile
from concourse import bacc
from concourse.bass_utils import run_bass_kernel_spmd

H = 4096
W = 4096
KH = 16
KW = 16
OH = H - KH + 1  # 4081
OW = W - KW + 1  # 4081
NCORES = 8

RPC = 512  # output rows per core (8*512 = 4096 >= 4081; tail trimmed)
SSTRIDE = 128  # strip stride
NSTRIPS = RPC // SSTRIDE  # 4
HALO = KH - 1  # 15
IN_ROWS = RPC + HALO  # 527 input rows needed per core
N_TILE = 512  # output cols per matmul (PSUM bank limit for fp32)

# column tiles: 7 full + ragged last computed at n0 = OW - N_TILE, storing
# only the non-overlapping tail (skip = overlap with tile 6)
N0_LIST = [(n0, 0) for n0 in range(0, OW - N_TILE + 1, N_TILE)]
_covered = (OW // N_TILE) * N_TILE
if _covered < OW:
    N0_LIST.append((OW - N_TILE, _covered - (OW - N_TILE)))
DUP_W = (OW - N_TILE) + 8 + N_TILE  # 4089; max col streamed from dup buffer

_DT_NAME = os.environ.get("CONV_MM_DTYPE", "float32r")
_OUT_DT_NAME = os.environ.get("CONV_OUT_DTYPE", "bfloat16")

_build_cache = {}


def _build(dt_name):
    loop = int(os.environ.get("CONV_LOOP", "1"))  # hw For_i loop (bench only)
    warm = int(os.environ.get("CONV_WARMUP_MM", "10"))
    psum_bufs = int(os.environ.get("CONV_PSUM_BUFS", "8"))
    strip_bufs = int(os.environ.get("CONV_STRIP_BUFS", "3"))
    out_dt_name = _OUT_DT_NAME
    dup_src = os.environ.get("CONV_DUP_SRC", "sbuf")
    key = (dt_name, out_dt_name, loop, warm, psum_bufs, strip_bufs, dup_src)
    if key in _build_cache:
        return _build_cache[key]
    DT = getattr(mybir.dt, dt_name)
    ODT = getattr(mybir.dt, out_dt_name)
    nc = bacc.Bacc()
    x_d = nc.dram_tensor("Xs", [IN_ROWS, W], DT, kind="ExternalInput")
    dup_d = nc.dram_tensor("dups", [NSTRIPS, 120, DUP_W], DT, kind="ExternalInput")
    wb_d = nc.dram_tensor("wband", [128, KW, 128], DT, kind="ExternalInput")
    wc_d = nc.dram_tensor("wcomp", [120, 2, 128], DT, kind="ExternalInput")
    bias_d = nc.dram_tensor("biasb", [128, 1], mybir.dt.float32, kind="ExternalInput")
    out_d = nc.dram_tensor("out", [RPC, OW], ODT, kind="ExternalOutput")

    # strip DMA split points: chunk j ends where column tile j's reads end, so
    # tile j's matmuls only wait on chunks <= j
    chunk_bounds = [0]
    for j in range(len(N0_LIST)):
        chunk_bounds.append(min((j + 1) * N_TILE + KW - 1, W))

    with tile.TileContext(nc) as tc:
        with (
            tc.tile_pool(name="const", bufs=1) as cpool,
            tc.tile_pool(name="strip", bufs=strip_bufs) as spool,
            tc.tile_pool(name="dup", bufs=NSTRIPS) as dpool,
            tc.tile_pool(name="obuf", bufs=3) as opool,
            tc.tile_pool(name="psum", bufs=psum_bufs, space="PSUM") as ppool,
        ):
            # PE warmup: dummy matmuls on a zeroed tile ramp the PE clock
            # while the first strip DMA is in flight
            if warm:
                # memset is invalid ISA for float32r tiles; allocate as plain
                # float32 and bitcast the matmul operands
                wsrc_dt = mybir.dt.float32 if dt_name == "float32r" else DT
                wsrc = cpool.tile([128, N_TILE], wsrc_dt)
                nc.vector.memset(wsrc[:, :], 0.0)
                for _ in range(warm):
                    wp = ppool.tile([128, N_TILE], mybir.dt.float32, tag="ps")
                    lhs = wsrc[:, :128]
                    rhs = wsrc[:, :N_TILE]
                    if wsrc_dt != DT:
                        lhs = lhs.bitcast(DT)
                        rhs = rhs.bitcast(DT)
                    nc.tensor.matmul(wp[:, :], lhs, rhs, start=True, stop=True)

            # constants: b=0 band slice first (gates the very first matmul);
            # bulk on the SWDGE queue so it doesn't delay strip chunks
            wb = cpool.tile([128, KW, 128], DT)
            nc.sync.dma_start(wb[:, 0:1, :], wb_d[:, 0:1, :])
            nc.gpsimd.dma_start(wb[:, 1:, :], wb_d[:, 1:, :])
            wcm = cpool.tile([120, 2, 128], DT)
            nc.gpsimd.dma_start(wcm[:], wc_d[:])
            bias_sb = cpool.tile([128, 1], mybir.dt.float32)
            nc.gpsimd.dma_start(bias_sb[:], bias_d[:])

            def body():
                # strip 0 chunked so the first matmuls aren't gated on 2MB
                strip0 = spool.tile([128, W], DT, tag="strip")
                for lo, hi in zip(chunk_bounds, chunk_bounds[1:]):
                    if hi > lo:
                        nc.sync.dma_start(strip0[:, lo:hi], x_d[0:128, lo:hi])

                strips = [strip0]
                for s in range(1, NSTRIPS):
                    st = spool.tile([128, W], DT, tag="strip")
                    nc.sync.dma_start(st[:, :], x_d[SSTRIDE * s : SSTRIDE * s + 128, :])
                    strips.append(st)

                # next-strip shifted duplicate buffers; emitted behind the
                # strip loads so they don't contend with the latency-critical
                # first chunks (first use is ~30us in)
                dups = []
                if dup_src == "sbuf":
                    # build dups from the already-resident strip tiles via
                    # SBUF->SBUF DMAs (no extra HBM traffic); only the
                    # core-boundary halo head comes from DRAM
                    headt = cpool.tile([HALO, W], DT)
                    nc.sync.dma_start(headt[:, :], x_d[RPC : RPC + HALO, :])
                    for s in range(NSTRIPS):
                        d = dpool.tile([120, DUP_W], DT, tag="dup")
                        src = strips[s + 1] if s < NSTRIPS - 1 else headt
                        for bp in range(8):
                            nc.scalar.dma_start(
                                d[bp * HALO : (bp + 1) * HALO, :],
                                src[0:HALO, bp : bp + DUP_W],
                            )
                        dups.append(d)
                else:
                    # prebuilt host-side, one DMA each from DRAM
                    for s in range(NSTRIPS):
                        d = dpool.tile([120, DUP_W], DT, tag="dup")
                        nc.sync.dma_start(d[:, :], dup_d[s])
                        dups.append(d)

                for s in range(NSTRIPS):
                    strip = strips[s]
                    dup = dups[s]
                    # batched output row-block: DVE writes segments, one store
                    ot = opool.tile([128, OW], ODT, tag="ot")
                    for n0, skip in N0_LIST:
                        ps = ppool.tile([128, N_TILE], mybir.dt.float32, tag="ps")
                        for b in range(KW):
                            nc.tensor.matmul(
                                ps[:, :],
                                wb[:, b, :],
                                strip[:, n0 + b : n0 + b + N_TILE],
                                start=(b == 0),
                                stop=False,
                            )
                        for h2 in range(2):
                            nc.tensor.matmul(
                                ps[:, :],
                                wcm[:, h2, :],
                                dup[:, n0 + 8 * h2 : n0 + 8 * h2 + N_TILE],
                                start=False,
                                stop=(h2 == 1),
                            )
                        nc.vector.tensor_scalar_add(
                            ot[:, n0 + skip : n0 + N_TILE],
                            ps[:, skip:N_TILE],
                            bias_sb[:, :],
                        )
                        if n0 == 3072:
                            # store the bulk once its segments are done; the
                            # final store is only the last ragged tile, so the
                            # kernel tail after the last matmul stays short
                            nc.scalar.dma_start(
                                out_d[SSTRIDE * s : SSTRIDE * s + 128, :3584],
                                ot[:, :3584],
                            )
                    nc.scalar.dma_start(
                        out_d[SSTRIDE * s : SSTRIDE * s + 128, 3584:], ot[:, 3584:]
                    )

            if loop > 1:
                with tc.For_i(0, loop, 1):
                    body()
            else:
                body()
    nc.finalize()
    _build_cache[key] = nc
    return nc


def make_in_maps(X, weight, bias, np_dt):
    pad_rows = NCORES * RPC + HALO  # 4111
    Xpad = np.zeros((pad_rows, W), dtype=np_dt)
    Xpad[:H] = X.astype(np_dt)

    wc = weight.astype(np_dt)
    # wband[r, b, m] = w[r - m, b] for 0 <= r-m < 16, truncated at r = 127
    wband = np.zeros((128, KW, 128), dtype=np_dt)
    for m in range(128):
        k = min(KH, 128 - m)
        wband[m : m + k, :, m] = wc[:k]
    # wcomp[bp*15 + r', h, m] = w[r' + 128 - m, bp + 8h] for taps falling in
    # the next strip (output rows m in [113, 127])
    wcomp = np.zeros((120, 2, 128), dtype=np_dt)
    for bp in range(8):
        for rp in range(HALO):
            p = bp * HALO + rp
            for h2 in range(2):
                b = bp + 8 * h2
                for m in range(rp + 113, 128):
                    wcomp[p, h2, m] = wc[rp + 128 - m, b]
    biasb = np.full((128, 1), np.float32(bias[0]), dtype=np.float32)

    in_maps = []
    for c in range(NCORES):
        xs = Xpad[c * RPC : c * RPC + IN_ROWS]
        dup = np.zeros((NSTRIPS, 120, DUP_W), dtype=np_dt)
        for s in range(NSTRIPS):
            base = SSTRIDE * (s + 1)
            for bp in range(8):
                dup[s, bp * HALO : (bp + 1) * HALO, :] = xs[
                    base : base + HALO, bp : bp + DUP_W
                ]
        in_maps.append(
            {
                "Xs": np.ascontiguousarray(xs),
                "dups": dup,
                "wband": wband,
                "wcomp": wcomp,
                "biasb": biasb,
            }
        )
    return in_maps


def _run(X, weight, bias, dt_name, trace=False):
    nc = _build(dt_name)
    np_dt = mybir.dt.np(getattr(mybir.dt, dt_name))
    in_maps = make_in_maps(X, weight, bias, np_dt)
    res = run_bass_kernel_spmd(
        nc, in_maps, core_ids=list(range(NCORES)), trace=trace
    )
    out = np.concatenate([res.results[c]["out"] for c in range(NCORES)], axis=0)
    return np.asarray(out[:OH], dtype=np.float32), res


def kernel(X, weight, bias):
    X = np.asarray(X, dtype=np.float32)
    weight = np.asarray(weight, dtype=np.float32)
    bias = np.asarray(bias, dtype=np.float32)
    out, _ = _run(X, weight, bias, _DT_NAME, trace=False)
    return out


# revision 28
# speedup vs baseline: 3.4033x; 2.5580x over previous
"""Trainium2 Bass kernel: 4096x4096 image, 16x16 valid cross-correlation + bias.

Strategy: shard output rows across 8 NeuronCores (spatial parallel, halo rows
overlapped host-side). Per core, the conv is computed as banded matmuls on
TensorE.

v2 scheme (stride-128 strips + cross-strip completion):
  - Strips advance by 128 rows (no overlap). The stationary banded matrix is
    [128 x 128]: column m holds taps w[0..15, b] at rows m..m+15, truncated at
    row 127. Rows m <= 112 of each PSUM tile are complete after the 16 main
    passes; rows 113..127 are missing the taps that fall into the next strip.
  - Two "completion" passes per column tile finish rows 113..127: the first 15
    rows of the next strip are duplicated into a [120 = 8 shifts x 15 rows]
    buffer D[bp*15+r', c] = Xnext[r', c+bp] (built on the idle Scalar engine),
    so one matmul pass covers 8 kernel columns at once; h in {0,1} covers
    kernel columns bp+8h.
  - Per core: 4 strips x 8 column tiles x 18 passes = 576 matmuls (vs 640 for
    the stride-113 scheme) and the per-core row count 512 = 4*128 divides
    evenly, so all cores do identical work (SPMD-balanced).
  - Dummy matmuls at t=0 ramp the PE clock (HAM warmup) while the first strip
    DMA lands, so real matmuls run at full rate from the start.

Matmul dtype float32r (fp32 storage, ~13-bit-mantissa multiplies, 1 cycle/row
for N>=256 — measured faster than bf16 on this device, which pays an
alignment penalty on 2-byte moving operands). Output is stored bf16 and
upconverted to fp32 host-side (l2 rel err ~1e-3, well under the 2e-2 gate).
"""
import os
import numpy as np

import concourse.mybir as mybir
import concourse.tile as tile
from concourse import bacc
from concourse.bass_utils import run_bass_kernel_spmd

H = 4096
W = 4096
KH = 16
KW = 16
OH = H - KH + 1  # 4081
OW = W - KW + 1  # 4081
NCORES = 8

RPC = 512  # output rows per core (8*512 = 4096 >= 4081; tail trimmed)
SSTRIDE = 128  # strip stride
NSTRIPS = RPC // SSTRIDE  # 4
HALO = KH - 1  # 15
IN_ROWS = RPC + HALO  # 527 input rows needed per core
N_TILE = 512  # output cols per matmul (PSUM bank limit for fp32)

# column tiles: 7 full + ragged last computed at n0 = OW - N_TILE, storing
# only the non-overlapping tail (skip = overlap with tile 6)
N0_LIST = [(n0, 0) for n0 in range(0, OW - N_TILE + 1, N_TILE)]
_covered = (OW // N_TILE) * N_TILE
if _covered < OW:
    N0_LIST.append((OW - N_TILE, _covered - (OW - N_TILE)))
DUP_W = (OW - N_TILE) + 8 + N_TILE  # 4089; max col streamed from dup buffer

_DT_NAME = os.environ.get("CONV_MM_DTYPE", "float32r")
_OUT_DT_NAME = os.environ.get("CONV_OUT_DTYPE", "bfloat16")

_build_cache = {}


def _build(dt_name):
    loop = int(os.environ.get("CONV_LOOP", "1"))  # hw For_i loop (bench only)
    warm = int(os.environ.get("CONV_WARMUP_MM", "10"))
    psum_bufs = int(os.environ.get("CONV_PSUM_BUFS", "8"))
    strip_bufs = int(os.environ.get("CONV_STRIP_BUFS", "4"))
    out_dt_name = _OUT_DT_NAME
    dup_src = os.environ.get("CONV_DUP_SRC", "sbuf")
    key = (dt_name, out_dt_name, loop, warm, psum_bufs, strip_bufs, dup_src)
    if key in _build_cache:
        return _build_cache[key]
    DT = getattr(mybir.dt, dt_name)
    ODT = getattr(mybir.dt, out_dt_name)
    nc = bacc.Bacc()
    x_d = nc.dram_tensor("Xs", [IN_ROWS, W], DT, kind="ExternalInput")
    dup_d = nc.dram_tensor("dups", [NSTRIPS, 120, DUP_W], DT, kind="ExternalInput")
    wb_d = nc.dram_tensor("wband", [128, KW, 128], DT, kind="ExternalInput")
    wc_d = nc.dram_tensor("wcomp", [120, 2, 128], DT, kind="ExternalInput")
    bias_d = nc.dram_tensor("biasb", [128, 1], mybir.dt.float32, kind="ExternalInput")
    out_d = nc.dram_tensor("out", [RPC, OW], ODT, kind="ExternalOutput")

    # strip DMA split points: chunk j ends where column tile j's reads end, so
    # tile j's matmuls only wait on chunks <= j
    chunk_bounds = [0]
    for j in range(len(N0_LIST)):
        chunk_bounds.append(min((j + 1) * N_TILE + KW - 1, W))

    with tile.TileContext(nc) as tc:
        with (
            tc.tile_pool(name="const", bufs=1) as cpool,
            tc.tile_pool(name="strip", bufs=strip_bufs) as spool,
            tc.tile_pool(name="dup", bufs=NSTRIPS) as dpool,
            tc.tile_pool(name="obuf", bufs=3) as opool,
            tc.tile_pool(name="psum", bufs=psum_bufs, space="PSUM") as ppool,
        ):
            # PE warmup: dummy matmuls on a zeroed tile ramp the PE clock
            # while the first strip DMA is in flight
            if warm:
                # memset is invalid ISA for float32r tiles; allocate as plain
                # float32 and bitcast the matmul operands
                wsrc_dt = mybir.dt.float32 if dt_name == "float32r" else DT
                wsrc = cpool.tile([128, N_TILE], wsrc_dt)
                nc.vector.memset(wsrc[:, :], 0.0)
                for _ in range(warm):
                    wp = ppool.tile([128, N_TILE], mybir.dt.float32, tag="ps")
                    lhs = wsrc[:, :128]
                    rhs = wsrc[:, :N_TILE]
                    if wsrc_dt != DT:
                        lhs = lhs.bitcast(DT)
                        rhs = rhs.bitcast(DT)
                    nc.tensor.matmul(wp[:, :], lhs, rhs, start=True, stop=True)

            # constants: b=0 band slice first (gates the very first matmul);
            # bulk on the SWDGE queue so it doesn't delay strip chunks
            wb = cpool.tile([128, KW, 128], DT)
            nc.sync.dma_start(wb[:, 0:1, :], wb_d[:, 0:1, :])
            nc.gpsimd.dma_start(wb[:, 1:, :], wb_d[:, 1:, :])
            wcm = cpool.tile([120, 2, 128], DT)
            nc.gpsimd.dma_start(wcm[:], wc_d[:])
            bias_sb = cpool.tile([128, 1], mybir.dt.float32)
            nc.gpsimd.dma_start(bias_sb[:], bias_d[:])

            def body():
                # strip 0 chunked so the first matmuls aren't gated on 2MB
                strip0 = spool.tile([128, W], DT, tag="strip")
                for lo, hi in zip(chunk_bounds, chunk_bounds[1:]):
                    if hi > lo:
                        nc.sync.dma_start(strip0[:, lo:hi], x_d[0:128, lo:hi])

                strips = [strip0]
                for s in range(1, NSTRIPS):
                    st = spool.tile([128, W], DT, tag="strip")
                    nc.sync.dma_start(st[:, :], x_d[SSTRIDE * s : SSTRIDE * s + 128, :])
                    strips.append(st)

                # next-strip shifted duplicate buffers; emitted behind the
                # strip loads so they don't contend with the latency-critical
                # first chunks (first use is ~30us in)
                dups = []
                if dup_src == "sbuf":
                    # build dups from the already-resident strip tiles via
                    # SBUF->SBUF DMAs (no extra HBM traffic); only the
                    # core-boundary halo head comes from DRAM
                    headt = cpool.tile([HALO, W], DT)
                    nc.sync.dma_start(headt[:, :], x_d[RPC : RPC + HALO, :])
                    for s in range(NSTRIPS):
                        d = dpool.tile([120, DUP_W], DT, tag="dup")
                        src = strips[s + 1] if s < NSTRIPS - 1 else headt
                        for bp in range(8):
                            nc.sync.dma_start(
                                d[bp * HALO : (bp + 1) * HALO, :],
                                src[0:HALO, bp : bp + DUP_W],
                            )
                        dups.append(d)
                else:
                    # prebuilt host-side, one DMA each from DRAM
                    for s in range(NSTRIPS):
                        d = dpool.tile([120, DUP_W], DT, tag="dup")
                        nc.sync.dma_start(d[:, :], dup_d[s])
                        dups.append(d)

                # output stores are chunked so the final store after the last
                # matmul is only the small ragged tile
                store_seg = {512: (0, 1024), 1536: (1024, 2048),
                             2560: (2048, 3072), 3072: (3072, 3584)}

                for s in range(NSTRIPS):
                    strip = strips[s]
                    dup = dups[s]
                    ot = opool.tile([128, OW], ODT, tag="ot")
                    rows = slice(SSTRIDE * s, SSTRIDE * s + 128)

                    def comp_and_consume(n0, skip, ps):
                        for h2 in range(2):
                            nc.tensor.matmul(
                                ps[:, :],
                                wcm[:, h2, :],
                                dup[:, n0 + 8 * h2 : n0 + 8 * h2 + N_TILE],
                                start=False,
                                stop=(h2 == 1),
                            )
                        # bias-add split across Vector and Scalar engines so
                        # the post-matmul drain at the kernel tail is halved
                        if (n0 // N_TILE) % 2 == 0:
                            nc.vector.tensor_scalar_add(
                                ot[:, n0 + skip : n0 + N_TILE],
                                ps[:, skip:N_TILE],
                                bias_sb[:, :],
                            )
                        else:
                            nc.scalar.add(
                                ot[:, n0 + skip : n0 + N_TILE],
                                ps[:, skip:N_TILE],
                                bias_sb[:, :],
                            )
                        seg = store_seg.get(n0)
                        if seg:
                            nc.gpsimd.dma_start(
                                out_d[rows, seg[0] : seg[1]], ot[:, seg[0] : seg[1]]
                            )

                    # strips 0..2: all main passes first (opens all 8 PSUM
                    # banks), then the completion passes — an in-order PE
                    # never waits for the dup buffers (first needed ~31us,
                    # ready ~16us). Last strip: interleave per tile so the
                    # kernel tail after the final matmul is minimal.
                    interleave = s == NSTRIPS - 1
                    ps_tiles = []
                    for n0, skip in N0_LIST:
                        ps = ppool.tile([128, N_TILE], mybir.dt.float32, tag="ps")
                        for b in range(KW):
                            nc.tensor.matmul(
                                ps[:, :],
                                wb[:, b, :],
                                strip[:, n0 + b : n0 + b + N_TILE],
                                start=(b == 0),
                                stop=False,
                            )
                        if interleave:
                            comp_and_consume(n0, skip, ps)
                        else:
                            ps_tiles.append((n0, skip, ps))
                    for n0, skip, ps in ps_tiles:
                        comp_and_consume(n0, skip, ps)
                    # final small store on the sync queue (idle at the tail;
                    # HWDGE issue is ~1.5us cheaper than SWDGE)
                    nc.sync.dma_start(out_d[rows, 3584:], ot[:, 3584:])

            if loop > 1:
                with tc.For_i(0, loop, 1):
                    body()
            else:
                body()
    nc.finalize()
    _build_cache[key] = nc
    return nc


def make_in_maps(X, weight, bias, np_dt):
    pad_rows = NCORES * RPC + HALO  # 4111
    Xpad = np.zeros((pad_rows, W), dtype=np_dt)
    Xpad[:H] = X.astype(np_dt)

    wc = weight.astype(np_dt)
    # wband[r, b, m] = w[r - m, b] for 0 <= r-m < 16, truncated at r = 127
    wband = np.zeros((128, KW, 128), dtype=np_dt)
    for m in range(128):
        k = min(KH, 128 - m)
        wband[m : m + k, :, m] = wc[:k]
    # wcomp[bp*15 + r', h, m] = w[r' + 128 - m, bp + 8h] for taps falling in
    # the next strip (output rows m in [113, 127])
    wcomp = np.zeros((120, 2, 128), dtype=np_dt)
    for bp in range(8):
        for rp in range(HALO):
            p = bp * HALO + rp
            for h2 in range(2):
                b = bp + 8 * h2
                for m in range(rp + 113, 128):
                    wcomp[p, h2, m] = wc[rp + 128 - m, b]
    biasb = np.full((128, 1), np.float32(bias[0]), dtype=np.float32)

    in_maps = []
    for c in range(NCORES):
        xs = Xpad[c * RPC : c * RPC + IN_ROWS]
        dup = np.zeros((NSTRIPS, 120, DUP_W), dtype=np_dt)
        for s in range(NSTRIPS):
            base = SSTRIDE * (s + 1)
            for bp in range(8):
                dup[s, bp * HALO : (bp + 1) * HALO, :] = xs[
                    base : base + HALO, bp : bp + DUP_W
                ]
        in_maps.append(
            {
                "Xs": np.ascontiguousarray(xs),
                "dups": dup,
                "wband": wband,
                "wcomp": wcomp,
                "biasb": biasb,
            }
        )
    return in_maps


def _run(X, weight, bias, dt_name, trace=False):
    nc = _build(dt_name)
    np_dt = mybir.dt.np(getattr(mybir.dt, dt_name))
    in_maps = make_in_maps(X, weight, bias, np_dt)
    res = run_bass_kernel_spmd(
        nc, in_maps, core_ids=list(range(NCORES)), trace=trace
    )
    out = np.concatenate([res.results[c]["out"] for c in range(NCORES)], axis=0)
    return np.asarray(out[:OH], dtype=np.float32), res


def kernel(X, weight, bias):
    X = np.asarray(X, dtype=np.float32)
    weight = np.asarray(weight, dtype=np.float32)
    bias = np.asarray(bias, dtype=np.float32)
    out, _ = _run(X, weight, bias, _DT_NAME, trace=False)
    return out
